# revision 37
# baseline (speedup 1.0000x reference)
"""DynaFormer (Graphormer-style GNN transformer) on 8 TRN2 NeuronCores.

Sharding: sequence-parallel over the N=2048 query rows (256/core).
Params replicated; K/V projections are eliminated by host-fusing
G = Wq Wk^T (scores via xn^T G xn) and M = Wv Wo (output via
M^T (xn^T P)); bk drops by softmax shift-invariance. Attention
probability tiles are fp8 (e4m3) and the P-accumulation (U = xn^T P)
and row-sum matmuls use fp8 DoubleRow (2x PE). Spatial bias is
computed as EB = exp(A(d) * P(t)), a degree-7 polynomial in
t = exp(d*step/sigma^2) (exact rewrite of the uniform Gaussian basis).
Degree histograms: one-hot masks on GpSimd + fp8 DoubleRow matmuls.
Collectives: 1 AllReduce (histograms, 16KB) + 2 AllGathers (512KB).
"""
import numpy as np
import ml_dtypes

import concourse.bass as bass
import concourse.bacc as bacc
import concourse.tile as tile
import concourse.mybir as mybir
from concourse.bass_utils import run_bass_kernel_spmd
from concourse.masks import make_identity

dt = mybir.dt
F32 = dt.float32
BF16 = dt.bfloat16
F8 = dt.float8e4
I32 = dt.int32
AF = mybir.ActivationFunctionType
ALU = mybir.AluOpType
DR = mybir.MatmulPerfMode.DoubleRow

N, E, IN_NODE, D = 2048, 65536, 16, 128
H, FFD, HS, L = 8, 512, 8, 3
MAXD, OUT = 64, 64
NC = 8
RPC = N // NC            # 256 rows/core
EPC = E // NC            # 8192 edges/core
NT = N // 128            # 16 node tiles
NG = EPC // 128          # 64 edge groups/core
SCALE = float(1.0 / np.sqrt(D))
LN16 = float(-4.0 * np.log(2.0))   # fold 1/16 into EB to keep fp8 in range

# packed weight segment layout: name -> (count, width)
# mwl/mwh: Wv@Wo row-halves, both packed at partitions 0:64 (DoubleRow
# U output lives at partition base 0, so the M-apply contraction is split)
_BSEGS = [("gt", 24, 128), ("mwl", 24, 128), ("mwh", 24, 128),
          ("w1", 3, 512), ("w2", 12, 128), ("ow", 1, 64)]
_FSEGS = [("rb", 24), ("l1g", 3), ("l1b", 3), ("l2g", 3), ("l2b", 3),
          ("b2c", 3), ("b1c", 12), ("bop", 3), ("ebc", 10)]
BOFF = {}
_o = 0
for _n, _c, _w in _BSEGS:
    BOFF[_n] = _o
    _o += _c * _w
NBF = _o
FOFF = {}
_o = 0
for _n, _c in _FSEGS:
    FOFF[_n] = _o
    _o += _c
NF32 = _o

_cached = {}

import os
U_FP8 = os.environ.get("KV_U_FP8", "1") == "1"   # fp8 xn (V-side) + DoubleRow U
PT_FP8 = os.environ.get("KV_PT_FP8", "1") == "1"  # fp8 attention probabilities


def bf16(x):
    return np.ascontiguousarray(np.asarray(x, np.float32).astype(ml_dtypes.bfloat16))


def f32(x):
    return np.ascontiguousarray(np.asarray(x, np.float32))


def build():
    nc = bacc.Bacc("TRN2", target_bir_lowering=False, debug=False,
                   enable_asserts=True, num_devices=NC)

    def din(name, shape, dty):
        return nc.dram_tensor(name, list(shape), dty, kind="ExternalInput").ap()

    def dout(name, shape, dty=F32):
        return nc.dram_tensor(name, list(shape), dty, kind="ExternalOutput").ap()

    # ---- dram params (common)
    xa_d = din("xa", [IN_NODE + 1, N], BF16)
    ztab2_d = din("ztab2", [128, D], BF16)      # [zout(64); zin(64)]
    posT_d = din("posT", [3, N], BF16)
    wb_d = din("wb", [128, NBF], BF16)
    pf_d = din("pf", [128, NF32], F32)
    wina_d = din("wina", [IN_NODE + 1, D], BF16)
    obc_d = din("obc", [OUT, 1], F32)
    io2_d = din("io2", [128, 1], F32)           # [0..63, 0..63] column
    # ---- per-core
    pcT_d = din("pcT", [3, RPC], BF16)
    src_d = din("src", [128, NG], I32)
    dst_d = din("dst", [128, NG], I32)
    oidx_d = din("oidx", [128, 2], I32)
    out_d = dout("out", [OUT, RPC], F32)
    DEBUG = os.environ.get("KV_DEBUG", "0") == "1"
    if DEBUG:
        dbg_xnT = dout("dbg_xnT", [D, N], F32)
        dbg_RT = dout("dbg_RT", [D, H * RPC], F32)
        dbg_U8n = dout("dbg_U8n", [128, 4 * 512], F32)
        dbg_rr = dout("dbg_rr", [1, 4 * 512], F32)
        dbg_oatT = dout("dbg_oatT", [D, RPC], F32)
        dbg_h1 = dout("dbg_h1", [128, 2 * D], F32)

    with tile.TileContext(nc) as tc:
        with tc.tile_pool(name="cp", bufs=1) as cp, \
             tc.tile_pool(name="wp", bufs=3) as wp, \
             tc.tile_pool(name="pp", bufs=2, space="PSUM") as pp, \
             tc.tile_pool(name="dp", bufs=1, space="DRAM") as dp:

            dma = nc.sync.dma_start

            # ---------- persistent constants ----------
            ident = cp.tile([128, 128], BF16)
            make_identity(nc, ident[:])
            eps5 = cp.tile([128, 1], F32)
            nc.vector.memset(eps5[:], 1e-5)
            eps12 = cp.tile([128, 1], F32)
            nc.vector.memset(eps12[:], 1e-12)
            ln16c = cp.tile([128, 1], F32)
            nc.vector.memset(ln16c[:], LN16)
            ones8 = cp.tile([128, 64], F8)
            nc.vector.memset(ones8[:], 1.0)

            WB = cp.tile([128, NBF], BF16)
            PF = cp.tile([128, NF32], F32)
            obc = cp.tile([OUT, 1], F32)

            def bseg(name, i, w):
                off = BOFF[name] + i * w
                return WB[:, off:off + w]

            def fseg(name, i):
                off = FOFF[name] + i
                return PF[:, off:off + 1]

            gt = [[bseg("gt", l * H + h, D) for h in range(H)] for l in range(L)]
            mwl = [[bseg("mwl", l * H + h, D) for h in range(H)] for l in range(L)]
            mwh = [[bseg("mwh", l * H + h, D) for h in range(H)] for l in range(L)]
            w1 = [bseg("w1", l, FFD) for l in range(L)]
            w2 = [[bseg("w2", l * 4 + fs, D) for fs in range(4)] for l in range(L)]
            ow = bseg("ow", 0, OUT)
            rb = [[fseg("rb", l * H + h) for h in range(H)] for l in range(L)]
            l1g = [fseg("l1g", l) for l in range(L)]
            l1b = [fseg("l1b", l) for l in range(L)]
            l2g = [fseg("l2g", l) for l in range(L)]
            l2b = [fseg("l2b", l) for l in range(L)]
            b2c = [fseg("b2c", l) for l in range(L)]
            b1c = [[fseg("b1c", l * 4 + fs) for fs in range(4)] for l in range(L)]
            bop = [fseg("bop", l) for l in range(L)]
            ebc = [fseg("ebc", i) for i in range(10)]  # c0..c7, kt, ahalf

            # persistent activations
            hown = [cp.tile([128, D], F32, name=f"ho{j}") for j in range(2)]
            EBdup = cp.tile([128, NT * 512], BF16)   # exp(bias)/16 dup per kt
            xnT = cp.tile([D, N], BF16)
            xn8 = cp.tile([128, N], F8 if U_FP8 else BF16)  # xn rows (key side)
            xnTo = cp.tile([D, RPC], BF16)
            RT = cp.tile([D, H * RPC], BF16)
            xn2T = cp.tile([D, RPC], BF16)
            gT = [cp.tile([128, RPC], BF16, name=f"gT{fs}") for fs in range(4)]
            htmp_dram = dp.tile([N, D], F32)
            ar_in = dp.tile([MAXD, 64], F32)
            ar_out = dp.tile([MAXD, 64], F32)
            ag_in = [dp.tile([D, RPC], BF16, name=f"agi{l}") for l in range(L - 1)]
            ag_out = [dp.tile([NC * D, RPC], BF16, name=f"ago{l}") for l in range(L - 1)]

            xn8r = xn8[:, :].rearrange("p (t d) -> p t d", t=NT)

            # ========================================================
            # PREPROC
            # ========================================================
            with tc.tile_pool(name="prep", bufs=1) as prp:
                posT = prp.tile([3, N], BF16)
                pcT = prp.tile([3, RPC], BF16)
                xa = prp.tile([IN_NODE + 1, N], BF16)
                wina = prp.tile([IN_NODE + 1, D], BF16)
                ztab2 = prp.tile([128, D], BF16)
                io2 = prp.tile([128, 1], F32)
                oidx = prp.tile([128, 2], I32)
                ones_bf = prp.tile([1, 128], BF16)
                nc.vector.memset(ones_bf[:], 1.0)
                ones3_f = prp.tile([3, 1], F32)
                nc.vector.memset(ones3_f[:], 1.0)

                # fast params first (pf holds EB consts needed immediately)
                dma(posT[:], posT_d[:])
                dma(pcT[:], pcT_d[:])
                dma(PF[:], pf_d[:])

                # ---------- p5/r5 for d^2 = |pk|^2 + |pq|^2 - 2 pk.pq ----------
                p5 = prp.tile([5, N], BF16)
                nc.vector.tensor_scalar(out=p5[0:3, :], in0=posT[:], scalar1=-2.0,
                                        scalar2=None, op0=ALU.mult)
                onesN = prp.tile([1, 512], BF16)
                nc.vector.memset(onesN[:], 1.0)
                for i in range(4):
                    dma(p5[4:5, i * 512:(i + 1) * 512], onesN[0:1, :])
                for i in range(4):
                    sq3 = wp.tile([3, 512], F32, tag="sq3", bufs=2)
                    nc.scalar.activation(sq3[:], posT[:, i * 512:(i + 1) * 512], AF.Square)
                    njp = pp.tile([1, 512], F32, space="PSUM", tag="rsp", bufs=1)
                    nc.tensor.matmul(out=njp[:], lhsT=ones3_f[:3, 0:1],
                                     rhs=sq3[:], start=True, stop=True)
                    njrow = wp.tile([1, 512], BF16, tag="njrow", bufs=2)
                    nc.vector.tensor_copy(njrow[:], njp[:])
                    dma(p5[3:4, i * 512:(i + 1) * 512], njrow[0:1, :])
                r5 = prp.tile([5, RPC], BF16)
                nc.vector.tensor_copy(r5[0:3, :], pcT[:])
                dma(r5[3:4, :], onesN[0:1, 0:RPC])
                sqc = wp.tile([3, RPC], F32, tag="sq3", bufs=2)
                nc.scalar.activation(sqc[:], pcT[:], AF.Square)
                nqp = pp.tile([1, RPC], F32, space="PSUM", tag="rsp", bufs=1)
                nc.tensor.matmul(out=nqp[:], lhsT=ones3_f[:3, 0:1], rhs=sqc[:],
                                 start=True, stop=True)
                nqrow = wp.tile([1, RPC], BF16, tag="njrow", bufs=2)
                nc.vector.tensor_copy(nqrow[:], nqp[:])
                dma(r5[4:5, :], nqrow[0:1, :])

                # rest of the param DMAs (queued behind the latency-critical few)
                dma(src_d_sb := prp.tile([128, NG], I32, name="srcsb"), src_d[:])
                dma(dst_d_sb := prp.tile([128, NG], I32, name="dstsb"), dst_d[:])
                dma(WB[:], wb_d[:])
                dma(obc[:], obc_d[:])
                dma(xa[:], xa_d[:])
                dma(wina[:], wina_d[:])
                dma(ztab2[:], ztab2_d[:])
                dma(io2[:], io2_d[:])
                dma(oidx[:], oidx_d[:])

                # ---------- d^2 matmuls -> dsq halves (PE + vector MAX) ----------
                HW_ = NT * RPC // 2
                dsq = [prp.tile([128, HW_], BF16, name=f"dsq{i}") for i in range(2)]
                for half in range(2):
                    for ki in range(NT // 2):
                        kt = half * (NT // 2) + ki
                        d2p = pp.tile([128, RPC], F32, space="PSUM", tag="stp", bufs=2)
                        nc.tensor.matmul(out=d2p[:],
                                         lhsT=p5[:, kt * 128:(kt + 1) * 128],
                                         rhs=r5[:], start=True, stop=True)
                        nc.vector.tensor_scalar(out=dsq[half][:, ki * RPC:(ki + 1) * RPC],
                                                in0=d2p[:], scalar1=0.0,
                                                scalar2=None, op0=ALU.max)

                # ---------- degree histogram masks on gpsimd ----------
                iot2 = prp.tile([128, 128], I32)
                nc.gpsimd.iota(iot2[:], pattern=[[0, 2], [1, 64]], base=0,
                               channel_multiplier=0)
                iot2f = prp.tile([128, 128], BF16)
                nc.gpsimd.tensor_copy(iot2f[:], iot2[:])
                io32 = prp.tile([128, 64], I32)
                nc.gpsimd.iota(io32[:], pattern=[[0, 2], [1, 32]], base=0,
                               channel_multiplier=0)
                io32f = prp.tile([128, 64], BF16)
                nc.gpsimd.tensor_copy(io32f[:], io32[:])
                # hist^T [32 quot, 64 rem] per dir via DoubleRow group-pairs
                hps = pp.tile([MAXD, 64], F32, space="PSUM", tag="oat", bufs=1)
                for di, ed in enumerate((src_d_sb, dst_d_sb)):
                    qi = wp.tile([128, NG], I32, tag="qi")
                    ri = wp.tile([128, NG], I32, tag="ri")
                    nc.vector.tensor_scalar(out=qi[:], in0=ed[:], scalar1=6,
                                            scalar2=None,
                                            op0=ALU.logical_shift_right)
                    nc.vector.tensor_scalar(out=ri[:], in0=ed[:], scalar1=63,
                                            scalar2=None, op0=ALU.bitwise_and)
                    qf = wp.tile([128, NG], BF16, tag="qf")
                    rf = wp.tile([128, NG], BF16, tag="rf")
                    nc.vector.tensor_copy(qf[:], qi[:])
                    nc.vector.tensor_copy(rf[:], ri[:])
                    for gp in range(NG // 2):
                        Bt = wp.tile([128, 128], F8, tag="Bt", bufs=3)
                        At = wp.tile([128, 64], F8, tag="At", bufs=3)
                        nc.vector.tensor_tensor(
                            out=Bt[:], in0=iot2f[:],
                            in1=rf[:, 2 * gp:2 * gp + 2].to_broadcast([128, 2, 64]),
                            op=ALU.is_equal)
                        nc.vector.tensor_tensor(
                            out=At[:], in0=io32f[:],
                            in1=qf[:, 2 * gp:2 * gp + 2].to_broadcast([128, 2, 32]),
                            op=ALU.is_equal)
                        for pl in range(2):
                            nc.tensor.matmul(
                                out=hps[di * 32:di * 32 + 32, :],
                                lhsT=At[:, pl * 32:(pl + 1) * 32],
                                rhs=Bt[:, pl * 64:(pl + 1) * 64],
                                start=(gp == 0 and pl == 0),
                                stop=(gp == NG // 2 - 1 and pl == 1),
                                skip_group_check=True)
                hsb = wp.tile([MAXD, 64], F32, tag="hsb", bufs=1)
                nc.scalar.activation(hsb[:], hps[:], AF.Identity)
                dma(ar_in[:, :], hsb[:])
                nc.gpsimd.collective_compute(
                    "AllReduce", ALU.add, replica_groups=[list(range(NC))],
                    ins=[ar_in.opt()], outs=[ar_out.opt()])

                # ---------- EB = exp(A * P(t)) / 16 (poly gaussian basis) ----------
                EBh = [prp.tile([128, HW_], BF16, name=f"EBh{i}") for i in range(2)]
                Ah = [prp.tile([128, HW_], BF16, name=f"Ah{i}") for i in range(2)]
                th = [prp.tile([128, HW_], BF16, name=f"th{i}") for i in range(2)]
                accA = [prp.tile([128, HW_], BF16, name=f"accA{i}") for i in range(2)]
                accB = [prp.tile([128, HW_], BF16, name=f"accB{i}") for i in range(2)]
                for half in range(2):
                    # scalar chain: A = exp(ahalf*d2), d = sqrt(d2), t = exp(kt*d)
                    nc.scalar.activation(Ah[half][:], dsq[half][:], AF.Exp,
                                         scale=ebc[9])
                    nc.scalar.activation(th[half][:], dsq[half][:], AF.Sqrt,
                                         bias=eps12[:, 0:1])
                    nc.scalar.activation(th[half][:], th[half][:], AF.Exp,
                                         scale=ebc[8])
                for half in range(2):
                    # vector Horner: acc = c7*t; acc = (acc + c_s)*t ...
                    nc.vector.tensor_scalar(out=accA[half][:], in0=th[half][:],
                                            scalar1=ebc[7],
                                            scalar2=None, op0=ALU.mult)
                    cur, nxt = accA[half], accB[half]
                    for s in range(6, 0, -1):
                        nc.vector.scalar_tensor_tensor(
                            out=nxt[:], in0=cur[:], scalar=ebc[s],
                            in1=th[half][:], op0=ALU.add, op1=ALU.mult)
                        cur, nxt = nxt, cur
                    # b = (acc + c0) * A
                    nc.vector.scalar_tensor_tensor(
                        out=nxt[:], in0=cur[:], scalar=ebc[0],
                        in1=Ah[half][:], op0=ALU.add, op1=ALU.mult)
                    # EB = exp(b)/16
                    nc.scalar.activation(EBh[half][:], nxt[:], AF.Exp,
                                         bias=ln16c[:, 0:1])
                    for ki in range(NT // 2):
                        kt = half * (NT // 2) + ki
                        dma(EBdup[:, kt * 512:kt * 512 + RPC],
                            EBh[half][:, ki * RPC:(ki + 1) * RPC])
                        dma(EBdup[:, kt * 512 + RPC:(kt + 1) * 512],
                            EBh[half][:, ki * RPC:(ki + 1) * RPC])

                # ---------- h0 + degree embeds (replicated) ----------
                arsb = prp.tile([MAXD, 64], F32)
                dma(arsb[:], ar_out[:, :])
                cmin = prp.tile([MAXD, 64], BF16)
                nc.vector.tensor_scalar(out=cmin[:], in0=arsb[:],
                                        scalar1=float(MAXD - 1), scalar2=None,
                                        op0=ALU.min)
                cntRow = prp.tile([1, 2 * N], BF16)
                dma(cntRow[0:1, 0:N], cmin[0:32, :])
                dma(cntRow[0:1, N:2 * N], cmin[32:64, :])
                hfull = [prp.tile([128, D], F32, name=f"hf{t}") for t in range(NT)]
                for t in range(NT):
                    bcps = pp.tile([128, 128], F32, space="PSUM", tag="psA", bufs=2)
                    for di in range(2):
                        nc.tensor.matmul(out=bcps[di * 64:(di + 1) * 64, :],
                                         lhsT=ones_bf[:1, 0:64],
                                         rhs=cntRow[0:1, di * N + t * 128:
                                                     di * N + (t + 1) * 128],
                                         start=True, stop=True,
                                         skip_group_check=True)
                    ohp = wp.tile([128, 128], BF16, tag="ohp", bufs=2)
                    nc.vector.tensor_scalar(out=ohp[:], in0=bcps[:],
                                            scalar1=io2[:, 0:1],
                                            scalar2=None, op0=ALU.is_equal)
                    hb = pp.tile([128, D], F32, space="PSUM",
                                 tag=("ups0" if t % 2 == 0 else "ups1"), bufs=1)
                    nc.tensor.matmul(out=hb[:], lhsT=xa[:, t * 128:(t + 1) * 128],
                                     rhs=wina[:], start=True, stop=False,
                                     skip_group_check=True)
                    nc.tensor.matmul(out=hb[:], lhsT=ohp[:], rhs=ztab2[:],
                                     start=False, stop=True,
                                     skip_group_check=True)
                    nc.vector.tensor_copy(hfull[t][:], hb[:])
                    dma(htmp_dram[t * 128:(t + 1) * 128, :], hfull[t][:])
                for j in range(2):
                    nc.gpsimd.indirect_dma_start(
                        out=hown[j][:], out_offset=None, in_=htmp_dram[:],
                        in_offset=bass.IndirectOffsetOnAxis(ap=oidx[:, j:j + 1], axis=0))

                # ---------- LN1 @ layer 0 (replicated, 18 tiles) ----------
                srcs = [hfull[t][:] for t in range(NT)] + [hown[j][:] for j in range(2)]
                outs = ([xnT[:, t * 128:(t + 1) * 128] for t in range(NT)]
                        + [xnTo[:, j * 128:(j + 1) * 128] for j in range(2)])
                nT18 = len(srcs)
                var = prp.tile([128, nT18], F32)
                mean = prp.tile([128, nT18], F32)
                for i, s_ap in enumerate(srcs):
                    st6 = wp.tile([128, 6], F32, tag="st6")
                    nc.vector.bn_stats(out=st6[:], in_=s_ap)
                    mv = wp.tile([128, 2], F32, tag="mv")
                    nc.vector.bn_aggr(out=mv[:], in_=st6[:])
                    nc.vector.tensor_copy(mean[:, i:i + 1], mv[:, 0:1])
                    nc.vector.tensor_copy(var[:, i:i + 1], mv[:, 1:2])
                sd = prp.tile([128, nT18], F32)
                nc.scalar.activation(sd[:], var[:], AF.Sqrt, bias=eps5[:, 0:1])
                rstd = prp.tile([128, nT18], F32)
                nc.vector.reciprocal(rstd[:], sd[:])
                for i, s_ap in enumerate(srcs):
                    xn = wp.tile([128, D], BF16, tag="xn")
                    nc.vector.tensor_scalar(out=xn[:], in0=s_ap, scalar1=mean[:, i:i + 1],
                                            scalar2=rstd[:, i:i + 1], op0=ALU.subtract,
                                            op1=ALU.mult)
                    trp = pp.tile([128, 128], BF16, space="PSUM", tag="psA")
                    nc.tensor.transpose(out=trp[:], in_=xn[:], identity=ident[:])
                    nc.vector.tensor_scalar(out=outs[i], in0=trp[:], scalar1=l1g[0],
                                            scalar2=l1b[0], op0=ALU.mult, op1=ALU.add)
                    if i < NT:
                        nc.vector.tensor_scalar(out=xn8[:, i * 128:(i + 1) * 128],
                                                in0=xn[:], scalar1=l1g[0],
                                                scalar2=l1b[0], op0=ALU.mult,
                                                op1=ALU.add)

            # ========================================================
            # LAYERS
            # ========================================================
            def ln_own(srcs, gcol, bcol, outs):
                """LayerNorm on own row-tiles -> transposed bf16 outputs."""
                nT = len(srcs)
                var = wp.tile([128, nT], F32, tag="lnvar")
                mean = wp.tile([128, nT], F32, tag="lnmean")
                for i, s_ap in enumerate(srcs):
                    st6 = wp.tile([128, 6], F32, tag="st6")
                    nc.vector.bn_stats(out=st6[:], in_=s_ap)
                    mv = wp.tile([128, 2], F32, tag="mv")
                    nc.vector.bn_aggr(out=mv[:], in_=st6[:])
                    nc.vector.tensor_copy(mean[:, i:i + 1], mv[:, 0:1])
                    nc.vector.tensor_copy(var[:, i:i + 1], mv[:, 1:2])
                sd = wp.tile([128, nT], F32, tag="lnsd")
                nc.scalar.activation(sd[:, :nT], var[:, :nT], AF.Sqrt, bias=eps5[:, 0:1])
                rstd = wp.tile([128, nT], F32, tag="lnrstd")
                nc.vector.reciprocal(rstd[:, :nT], sd[:, :nT])
                for i, s_ap in enumerate(srcs):
                    xn = wp.tile([128, D], BF16, tag="xn")
                    nc.vector.tensor_scalar(out=xn[:], in0=s_ap, scalar1=mean[:, i:i + 1],
                                            scalar2=rstd[:, i:i + 1], op0=ALU.subtract,
                                            op1=ALU.mult)
                    trp = pp.tile([128, 128], BF16, space="PSUM", tag="psA")
                    nc.tensor.transpose(out=trp[:], in_=xn[:], identity=ident[:])
                    nc.vector.tensor_scalar(out=outs[i], in0=trp[:], scalar1=gcol,
                                            scalar2=bcol, op0=ALU.mult, op1=ALU.add)

            hpt = [wp.tile([128, D], F32, name=f"hp{j}") for j in range(2)]

            for l in range(L):
                # R = (Wk Wq^T) xn_own^T + Wk bq, per head (pre-AllGather work)
                for h in range(H):
                    rp = pp.tile([D, RPC], F32, space="PSUM", tag="psA")
                    nc.tensor.matmul(out=rp[:], lhsT=gt[l][h], rhs=xnTo[:],
                                     start=True, stop=True)
                    nc.vector.tensor_scalar(out=RT[:, h * RPC:(h + 1) * RPC],
                                            in0=rp[:], scalar1=rb[l][h],
                                            scalar2=None, op0=ALU.add)
                # xn8 from gathered xnT (keys in rows, fp8)
                if l > 0:
                    for t in range(NT):
                        trp = pp.tile([128, 128], BF16, space="PSUM", tag="psA")
                        nc.tensor.transpose(out=trp[:],
                                            in_=xnT[:, t * 128:(t + 1) * 128],
                                            identity=ident[:])
                        nc.vector.tensor_copy(xn8[:, t * 128:(t + 1) * 128], trp[:])

                # flash attention, two heads per pass, fp8 DoubleRow PV
                oat = pp.tile([D, RPC], F32, space="PSUM", tag="oat", bufs=1)
                deferred_pe = [None]

                def flush_deferred():
                    if deferred_pe[0] is not None:
                        deferred_pe[0]()
                        deferred_pe[0] = None

                for hp2 in range(H // 2):
                    h0, h1 = 2 * hp2, 2 * hp2 + 1
                    # DoubleRow outputs must sit at partition base 0, so
                    # U[d, q] per head lives as [64, (dh, q)]; the second
                    # column-block group uses start=False and inherits the
                    # bank-wide pending-zero set by the first group's start.
                    Ups = [pp.tile([64, 512], F32, space="PSUM", tag=f"ups{jj}",
                                   bufs=1, name=f"ups{jj}") for jj in range(2)]
                    rsp = pp.tile([32, 512], F32, space="PSUM", tag="rsp", bufs=1)

                    def scores_exp(kt, PT8cur, j):
                        stp = pp.tile([128, 512], F32, space="PSUM", tag="stp",
                                      bufs=2)
                        nc.tensor.matmul(out=stp[:, 0:RPC],
                                         lhsT=xnT[:, kt * 128:(kt + 1) * 128],
                                         rhs=RT[:, h0 * RPC:(h0 + 1) * RPC],
                                         start=True, stop=True,
                                         skip_group_check=True)
                        nc.tensor.matmul(out=stp[:, RPC:2 * RPC],
                                         lhsT=xnT[:, kt * 128:(kt + 1) * 128],
                                         rhs=RT[:, h1 * RPC:(h1 + 1) * RPC],
                                         start=True, stop=True,
                                         skip_group_check=True)
                        PTt = wp.tile([128, 512], BF16, tag="PTt", bufs=3)
                        nc.scalar.activation(PTt[:], stp[:], AF.Exp, scale=SCALE)
                        nc.gpsimd.tensor_tensor(
                            out=PT8cur[:, j * 512:(j + 1) * 512], in0=PTt[:],
                            in1=EBdup[:, kt * 512:(kt + 1) * 512], op=ALU.mult)

                    for t in range(NT // 2):
                        PT8cur = wp.tile([128, 1024], F8 if PT_FP8 else BF16,
                                         tag="PT8", bufs=2)
                        scores_exp(2 * t, PT8cur, 0)
                        scores_exp(2 * t + 1, PT8cur, 1)
                        if t == 0:
                            flush_deferred()
                        p8r = PT8cur[:, :].rearrange("p (j q) -> p j q", j=2)
                        for jj in range(2):
                            rhs = p8r[:, :, jj * RPC:(jj + 1) * RPC]
                            for dh in range(2):
                                nc.tensor.matmul(
                                    out=Ups[jj][0:64, dh * RPC:(dh + 1) * RPC],
                                    lhsT=xn8r[:, 2 * t:2 * t + 2,
                                              dh * 64:(dh + 1) * 64],
                                    rhs=rhs, start=(t == 0 and dh == 0),
                                    stop=(t == NT // 2 - 1),
                                    perf_mode=DR, skip_group_check=True)
                            nc.tensor.matmul(
                                out=rsp[0:32, jj * RPC:(jj + 1) * RPC],
                                lhsT=ones8[:, :].rearrange(
                                    "p (t m) -> p t m", t=2),
                                rhs=rhs,
                                start=(t == 0 and jj == 0),
                                stop=(t == NT // 2 - 1),
                                perf_mode=DR, skip_group_check=True)

                    # epilogue: normalize U per head, apply fused M = Wv Wo
                    rrow = wp.tile([1, 1024], F32, tag="rrow", bufs=2)
                    U8n = wp.tile([64, 1024], BF16, tag="U8n", bufs=2)
                    rbc = wp.tile([64, 1024], F32, tag="rbc", bufs=2)
                    for jj in range(2):
                        # duplicate the reciprocal row for both d-halves
                        nc.vector.reciprocal(rrow[0:1, jj * 512:jj * 512 + RPC],
                                             rsp[0:1, jj * RPC:(jj + 1) * RPC])
                        nc.vector.tensor_copy(
                            rrow[0:1, jj * 512 + RPC:(jj + 1) * 512],
                            rrow[0:1, jj * 512:jj * 512 + RPC])
                        nc.gpsimd.partition_broadcast(
                            rbc[:, jj * 512:(jj + 1) * 512],
                            rrow[0:1, jj * 512:(jj + 1) * 512])
                        nc.vector.tensor_tensor(
                            out=U8n[:, jj * 512:(jj + 1) * 512],
                            in0=Ups[jj][0:64, :],
                            in1=rbc[:, jj * 512:(jj + 1) * 512], op=ALU.mult)

                    def mk_mapply(hp2=hp2, h0=h0, h1=h1, U8n=U8n):
                        def go():
                            for jj, hh in ((0, h0), (1, h1)):
                                for dh, mwseg in ((0, mwl), (1, mwh)):
                                    nc.tensor.matmul(
                                        out=oat[:],
                                        lhsT=mwseg[l][hh][0:64, :],
                                        rhs=U8n[0:64, jj * 512 + dh * RPC:
                                                jj * 512 + (dh + 1) * RPC],
                                        start=(hp2 == 0 and jj == 0 and dh == 0),
                                        stop=(hp2 == H // 2 - 1 and jj == 1
                                              and dh == 1),
                                        skip_group_check=True)
                        return go
                    deferred_pe[0] = mk_mapply()
                flush_deferred()

                # attention out: + bop, transpose to rows, residual
                oatT = wp.tile([D, RPC], BF16, tag="oatT")
                nc.vector.tensor_scalar(out=oatT[:], in0=oat[:], scalar1=bop[l],
                                        scalar2=None, op0=ALU.add)
                if DEBUG and l == 0:
                    o32 = wp.tile([D, RPC], F32, tag="dbgu", bufs=2)
                    nc.vector.tensor_copy(o32[:], oatT[:])
                    dma(dbg_oatT[:, :], o32[:])
                    x32 = wp.tile([D, RPC], F32, tag="dbgu", bufs=2)
                    for tt in range(NT):
                        nc.vector.tensor_copy(x32[:, 0:128], xnT[:, tt * 128:(tt + 1) * 128])
                        dma(dbg_xnT[:, tt * 128:(tt + 1) * 128], x32[:, 0:128])
                    r32 = wp.tile([D, RPC], F32, tag="dbgu", bufs=2)
                    for hh in range(H):
                        nc.vector.tensor_copy(r32[:], RT[:, hh * RPC:(hh + 1) * RPC])
                        dma(dbg_RT[:, hh * RPC:(hh + 1) * RPC], r32[:])
                for j in range(2):
                    ftp = pp.tile([128, 128], BF16, space="PSUM", tag="psA")
                    nc.tensor.transpose(out=ftp[:], in_=oatT[:, j * 128:(j + 1) * 128],
                                        identity=ident[:])
                    nc.vector.tensor_tensor(out=hpt[j][:], in0=hown[j][:],
                                            in1=ftp[:], op=ALU.add)
                # LN2 + FF on own rows
                ln_own([hpt[j][:] for j in range(2)], l2g[l], l2b[l],
                       [xn2T[:, j * 128:(j + 1) * 128] for j in range(2)])
                for fs in range(4):
                    fp = pp.tile([128, RPC], F32, space="PSUM", tag="psA")
                    nc.tensor.matmul(out=fp[:], lhsT=w1[l][:, fs * 128:(fs + 1) * 128],
                                     rhs=xn2T[:], start=True, stop=True)
                    nc.scalar.activation(gT[fs][:], fp[:],
                                         AF.Gelu, bias=b1c[l][fs])
                fdp = pp.tile([D, RPC], F32, space="PSUM", tag="stp", bufs=2)
                for fs in range(4):
                    nc.tensor.matmul(out=fdp[:], lhsT=w2[l][fs],
                                     rhs=gT[fs][:],
                                     start=(fs == 0), stop=(fs == 3))
                ffdT = wp.tile([D, RPC], BF16, tag="ffdT")
                nc.vector.tensor_scalar(out=ffdT[:], in0=fdp[:], scalar1=b2c[l],
                                        scalar2=None, op0=ALU.add)
                for j in range(2):
                    ftp = pp.tile([128, 128], BF16, space="PSUM", tag="psA")
                    nc.tensor.transpose(out=ftp[:], in_=ffdT[:, j * 128:(j + 1) * 128],
                                        identity=ident[:])
                    nc.vector.tensor_tensor(out=hown[j][:], in0=hpt[j][:], in1=ftp[:],
                                            op=ALU.add)
                if DEBUG and l == 0:
                    for j in range(2):
                        dma(dbg_h1[:, j * D:(j + 1) * D], hown[j][:])
                # LN1 for next layer on own rows; AllGather xn^T (bf16)
                if l < L - 1:
                    ln_own([hown[j][:] for j in range(2)], l1g[l + 1], l1b[l + 1],
                           [xnTo[:, j * 128:(j + 1) * 128] for j in range(2)])
                    dma(ag_in[l][:, :], xnTo[:])
                    nc.gpsimd.collective_compute(
                        "AllGather", ALU.bypass, replica_groups=[list(range(NC))],
                        ins=[ag_in[l].opt()], outs=[ag_out[l].opt()])
                    for c in range(NC):
                        dma(xnT[:, c * RPC:(c + 1) * RPC],
                            ag_out[l][c * D:(c + 1) * D, :])

            # ================= output =================
            for j in range(2):
                hb16 = wp.tile([128, D], BF16, tag="hb16")
                nc.vector.tensor_copy(hb16[:], hown[j][:])
                htp = pp.tile([128, 128], BF16, space="PSUM", tag="psA")
                nc.tensor.transpose(out=htp[:], in_=hb16[:], identity=ident[:])
                hT = wp.tile([D, 128], BF16, tag="hT")
                nc.vector.tensor_copy(hT[:], htp[:])
                op_ps = pp.tile([OUT, 128], F32, space="PSUM", tag="psA")
                nc.tensor.matmul(out=op_ps[:], lhsT=ow, rhs=hT[:],
                                 start=True, stop=True)
                ob_sb = wp.tile([OUT, 128], F32, tag="ob_sb")
                nc.scalar.activation(ob_sb[:], op_ps[:], AF.Identity,
                                     bias=obc[:, 0:1])
                dma(out_d[:, j * 128:(j + 1) * 128], ob_sb[:])

    nc.finalize()
    return nc


def _pack_bf16(I):
    Wq, Wk, Wo_, Wv_ = f32(I["Wq"]), f32(I["Wk"]), f32(I["Wo"]), f32(I["Wv"])
    cols = []
    for l in range(L):
        for h in range(H):
            cols.append(Wq[l, h] @ Wk[l, h].T)          # gt: lhsT for R
    mwfull = [[Wv_[l, h] @ Wo_[l, h * D:(h + 1) * D, :] for h in range(H)]
              for l in range(L)]
    z64 = np.zeros((64, D), np.float32)
    for l in range(L):
        for h in range(H):
            cols.append(np.concatenate([mwfull[l][h][0:64], z64], 0))   # mwl
    for l in range(L):
        for h in range(H):
            cols.append(np.concatenate([mwfull[l][h][64:128], z64], 0))  # mwh
    for l in range(L):
        cols.append(f32(I["ff1_w"])[l])
    for l in range(L):
        for fs in range(4):
            cols.append(f32(I["ff2_w"])[l, fs * 128:(fs + 1) * 128, :])
    cols.append(f32(I["out_w"]))
    out = np.concatenate(cols, 1)
    assert out.shape == (128, NBF), out.shape
    return bf16(out)


def _pack_f32(I):
    Wk, Wo_ = f32(I["Wk"]), f32(I["Wo"])
    cols = []
    for l in range(L):
        for h in range(H):
            cols.append((Wk[l, h] @ f32(I["bq"])[l, h])[:, None])  # rb
    for nm in ("ln1_g", "ln1_b", "ln2_g", "ln2_b"):
        for l in range(L):
            cols.append(f32(I[nm])[l][:, None])
    for l in range(L):
        cols.append(f32(I["ff2_b"])[l][:, None])
    for l in range(L):
        for fs in range(4):
            cols.append(f32(I["ff1_b"])[l, fs * 128:(fs + 1) * 128][:, None])
    for l in range(L):
        bo_l = f32(I["bo"])[l] + sum(
            f32(I["bv"])[l, h] @ Wo_[l, h * D:(h + 1) * D, :] for h in range(H))
        cols.append(bo_l[:, None])
    # EB poly constants: c0..c7, kt, ahalf (broadcast down partitions)
    mu = f32(I["sp_mu"]); sg = f32(I["sp_sigma"]); w = f32(I["sp_w"])
    s2 = float(sg[0]) ** 2
    step = float(mu[1] - mu[0])
    c = w * np.exp(-0.5 * mu * mu / s2)
    ebvals = list(c.astype(np.float64)) + [step / s2, -0.5 / s2]
    for v in ebvals:
        cols.append(np.full((128, 1), v, np.float32))
    out = np.concatenate(cols, 1)
    assert out.shape == (128, NF32), out.shape
    return f32(out)


def _prep(inputs):
    I = {k: np.asarray(v) for k, v in inputs.items()}
    x = f32(I["x"])
    pos = f32(I["pos"])
    ei = I["edge_index"].astype(np.int32)

    common = {
        "xa": bf16(np.concatenate([x.T, np.ones((1, N), np.float32)], 0)),
        "wina": bf16(np.concatenate([f32(I["node_in_w"]), f32(I["node_in_b"])[None]], 0)),
        "ztab2": bf16(np.concatenate([f32(I["z_out"]), f32(I["z_in"])], 0)),
        "posT": bf16(pos.T),
        "wb": _pack_bf16(I),
        "pf": _pack_f32(I),
        "obc": f32(I["out_b"])[:, None],
        "io2": np.concatenate([np.arange(MAXD, dtype=np.float32)] * 2)[:, None],
    }
    common = {k: np.ascontiguousarray(v) for k, v in common.items()}
    in_maps = []
    for c in range(NC):
        m = dict(common)
        qsl = slice(c * RPC, (c + 1) * RPC)
        m["pcT"] = bf16(pos[qsl].T)
        m["src"] = np.ascontiguousarray(
            ei[0, c * EPC:(c + 1) * EPC].reshape(NG, 128).T)
        m["dst"] = np.ascontiguousarray(
            ei[1, c * EPC:(c + 1) * EPC].reshape(NG, 128).T)
        m["oidx"] = np.ascontiguousarray(
            (c * RPC + np.arange(RPC, dtype=np.int32)).reshape(2, 128).T)
        in_maps.append(m)
    return in_maps


def kernel(**inputs) -> np.ndarray:
    if "nc" not in _cached:
        _cached["nc"] = build()
    in_maps = _prep(inputs)
    res = run_bass_kernel_spmd(_cached["nc"], in_maps, core_ids=list(range(NC)))
    _cached["last_results"] = res
    out = np.concatenate([f32(r["out"]).T for r in res.results], 0)
    return out.astype(np.float32)


# revision 39
# speedup vs baseline: 1.7479x; 1.7479x over previous
"""DynaFormer (Graphormer-style GNN transformer) on 8 TRN2 NeuronCores.

Sharding: sequence-parallel over the N=2048 query rows (256/core).
Params replicated; K/V projections are eliminated by host-fusing
G = Wq Wk^T (scores via xn^T G xn) and M = Wv Wo (output via
M^T (xn^T P)); bk drops by softmax shift-invariance. Attention
probability tiles are fp8 (e4m3) and the P-accumulation (U = xn^T P)
and row-sum matmuls use fp8 DoubleRow (2x PE). Spatial bias is
computed as EB = exp(A(d) * P(t)), a degree-7 polynomial in
t = exp(d*step/sigma^2) (exact rewrite of the uniform Gaussian basis).
Degree histograms: one-hot masks on GpSimd + fp8 DoubleRow matmuls.
Collectives: 1 AllReduce (histograms, 16KB) + 2 AllGathers (512KB).
"""
import numpy as np
import ml_dtypes

import concourse.bass as bass
import concourse.bacc as bacc
import concourse.tile as tile
import concourse.mybir as mybir
from concourse.bass_utils import run_bass_kernel_spmd
from concourse.masks import make_identity

dt = mybir.dt
F32 = dt.float32
BF16 = dt.bfloat16
F8 = dt.float8e4
I32 = dt.int32
AF = mybir.ActivationFunctionType
ALU = mybir.AluOpType
DR = mybir.MatmulPerfMode.DoubleRow

N, E, IN_NODE, D = 2048, 65536, 16, 128
H, FFD, HS, L = 8, 512, 8, 3
MAXD, OUT = 64, 64
NC = 8
RPC = N // NC            # 256 rows/core
EPC = E // NC            # 8192 edges/core
NT = N // 128            # 16 node tiles
NG = EPC // 128          # 64 edge groups/core
SCALE = float(1.0 / np.sqrt(D))
LN16 = float(-4.0 * np.log(2.0))   # fold 1/16 into EB to keep fp8 in range

# packed weight segment layout: name -> (count, width)
# mwl/mwh: Wv@Wo row-halves, both packed at partitions 0:64 (DoubleRow
# U output lives at partition base 0, so the M-apply contraction is split)
_BSEGS = [("gt", 24, 128), ("mwl", 24, 128), ("mwh", 24, 128),
          ("w1", 3, 512), ("w2", 12, 128), ("ow", 1, 64)]
_FSEGS = [("rb", 24), ("l1g", 3), ("l1b", 3), ("l2g", 3), ("l2b", 3),
          ("b2c", 3), ("b1c", 12), ("bop", 3), ("ebc", 10)]
BOFF = {}
_o = 0
for _n, _c, _w in _BSEGS:
    BOFF[_n] = _o
    _o += _c * _w
NBF = _o
FOFF = {}
_o = 0
for _n, _c in _FSEGS:
    FOFF[_n] = _o
    _o += _c
NF32 = _o

_cached = {}

import os
U_FP8 = os.environ.get("KV_U_FP8", "1") == "1"   # fp8 xn (V-side) + DoubleRow U
PT_FP8 = os.environ.get("KV_PT_FP8", "1") == "1"  # fp8 attention probabilities


def bf16(x):
    return np.ascontiguousarray(np.asarray(x, np.float32).astype(ml_dtypes.bfloat16))


def f32(x):
    return np.ascontiguousarray(np.asarray(x, np.float32))


def build():
    nc = bacc.Bacc("TRN2", target_bir_lowering=False, debug=False,
                   enable_asserts=True, num_devices=NC)

    def din(name, shape, dty):
        return nc.dram_tensor(name, list(shape), dty, kind="ExternalInput").ap()

    def dout(name, shape, dty=F32):
        return nc.dram_tensor(name, list(shape), dty, kind="ExternalOutput").ap()

    # ---- dram params (common)
    xa_d = din("xa", [IN_NODE + 1, N], BF16)
    ztab2_d = din("ztab2", [128, D], BF16)      # [zout(64); zin(64)]
    posT_d = din("posT", [3, N], BF16)
    wb_d = din("wb", [128, NBF], BF16)
    pf_d = din("pf", [128, NF32], F32)
    wina_d = din("wina", [IN_NODE + 1, D], BF16)
    obc_d = din("obc", [OUT, 1], F32)
    io2_d = din("io2", [128, 1], F32)           # [0..63, 0..63] column
    # ---- per-core
    pcT_d = din("pcT", [3, RPC], BF16)
    src_d = din("src", [128, NG], I32)
    dst_d = din("dst", [128, NG], I32)
    oidx_d = din("oidx", [128, 2], I32)
    out_d = dout("out", [OUT, RPC], F32)
    DEBUG = os.environ.get("KV_DEBUG", "0") == "1"
    if DEBUG:
        dbg_xnT = dout("dbg_xnT", [D, N], F32)
        dbg_RT = dout("dbg_RT", [D, H * RPC], F32)
        dbg_U8n = dout("dbg_U8n", [128, 4 * 512], F32)
        dbg_rr = dout("dbg_rr", [1, 4 * 512], F32)
        dbg_oatT = dout("dbg_oatT", [D, RPC], F32)
        dbg_h1 = dout("dbg_h1", [128, 2 * D], F32)

    with tile.TileContext(nc) as tc:
        with tc.tile_pool(name="cp", bufs=1) as cp, \
             tc.tile_pool(name="wp", bufs=3) as wp, \
             tc.tile_pool(name="pp", bufs=2, space="PSUM") as pp, \
             tc.tile_pool(name="dp", bufs=1, space="DRAM") as dp:

            dma = nc.sync.dma_start

            # ---------- persistent constants ----------
            ident = cp.tile([128, 128], BF16)
            make_identity(nc, ident[:])
            eps5 = cp.tile([128, 1], F32)
            nc.vector.memset(eps5[:], 1e-5)
            eps12 = cp.tile([128, 1], F32)
            nc.vector.memset(eps12[:], 1e-12)
            ln16c = cp.tile([128, 1], F32)
            nc.vector.memset(ln16c[:], LN16)
            ones8 = cp.tile([128, 64], F8)
            nc.vector.memset(ones8[:], 1.0)

            WB = cp.tile([128, NBF], BF16)
            PF = cp.tile([128, NF32], F32)
            obc = cp.tile([OUT, 1], F32)

            def bseg(name, i, w):
                off = BOFF[name] + i * w
                return WB[:, off:off + w]

            def fseg(name, i):
                off = FOFF[name] + i
                return PF[:, off:off + 1]

            gt = [[bseg("gt", l * H + h, D) for h in range(H)] for l in range(L)]
            mwl = [[bseg("mwl", l * H + h, D) for h in range(H)] for l in range(L)]
            mwh = [[bseg("mwh", l * H + h, D) for h in range(H)] for l in range(L)]
            w1 = [bseg("w1", l, FFD) for l in range(L)]
            w2 = [[bseg("w2", l * 4 + fs, D) for fs in range(4)] for l in range(L)]
            ow = bseg("ow", 0, OUT)
            rb = [[fseg("rb", l * H + h) for h in range(H)] for l in range(L)]
            l1g = [fseg("l1g", l) for l in range(L)]
            l1b = [fseg("l1b", l) for l in range(L)]
            l2g = [fseg("l2g", l) for l in range(L)]
            l2b = [fseg("l2b", l) for l in range(L)]
            b2c = [fseg("b2c", l) for l in range(L)]
            b1c = [[fseg("b1c", l * 4 + fs) for fs in range(4)] for l in range(L)]
            bop = [fseg("bop", l) for l in range(L)]
            ebc = [fseg("ebc", i) for i in range(10)]  # c0..c7, kt, ahalf

            # persistent activations
            hown = [cp.tile([128, D], F32, name=f"ho{j}") for j in range(2)]
            EBdup = cp.tile([128, NT * 512], BF16)   # exp(bias)/16 dup per kt
            xnT = cp.tile([D, N], BF16)
            xn8 = cp.tile([128, N], F8 if U_FP8 else BF16)  # xn rows (key side)
            xnTo = cp.tile([D, RPC], BF16)
            RT = cp.tile([D, H * RPC], BF16)
            xn2T = cp.tile([D, RPC], BF16)
            gT = [cp.tile([128, RPC], BF16, name=f"gT{fs}") for fs in range(4)]
            htmp_dram = dp.tile([N, D], F32)
            ar_in = dp.tile([MAXD, 64], F32)
            ar_out = dp.tile([MAXD, 64], F32)
            ag_in = [dp.tile([D, RPC], BF16, name=f"agi{l}") for l in range(L - 1)]
            ag_out = [dp.tile([NC * D, RPC], BF16, name=f"ago{l}") for l in range(L - 1)]

            xn8r = xn8[:, :].rearrange("p (t d) -> p t d", t=NT)

            # ========================================================
            # PREPROC
            # ========================================================
            with tc.tile_pool(name="prep", bufs=1) as prp:
                posT = prp.tile([3, N], BF16)
                pcT = prp.tile([3, RPC], BF16)
                xa = prp.tile([IN_NODE + 1, N], BF16)
                wina = prp.tile([IN_NODE + 1, D], BF16)
                ztab2 = prp.tile([128, D], BF16)
                io2 = prp.tile([128, 1], F32)
                oidx = prp.tile([128, 2], I32)
                ones_bf = prp.tile([1, 128], BF16)
                nc.vector.memset(ones_bf[:], 1.0)
                ones3_f = prp.tile([3, 1], F32)
                nc.vector.memset(ones3_f[:], 1.0)

                # fast params first (pf holds EB consts needed immediately)
                dma(posT[:], posT_d[:])
                dma(pcT[:], pcT_d[:])
                dma(PF[:], pf_d[:])

                # ---------- p5/r5 for d^2 = |pk|^2 + |pq|^2 - 2 pk.pq ----------
                p5 = prp.tile([5, N], BF16)
                nc.vector.tensor_scalar(out=p5[0:3, :], in0=posT[:], scalar1=-2.0,
                                        scalar2=None, op0=ALU.mult)
                onesN = prp.tile([1, 512], BF16)
                nc.vector.memset(onesN[:], 1.0)
                for i in range(4):
                    dma(p5[4:5, i * 512:(i + 1) * 512], onesN[0:1, :])
                for i in range(4):
                    sq3 = wp.tile([3, 512], F32, tag="sq3", bufs=2)
                    nc.scalar.activation(sq3[:], posT[:, i * 512:(i + 1) * 512], AF.Square)
                    njp = pp.tile([1, 512], F32, space="PSUM", tag="rsp", bufs=1)
                    nc.tensor.matmul(out=njp[:], lhsT=ones3_f[:3, 0:1],
                                     rhs=sq3[:], start=True, stop=True)
                    njrow = wp.tile([1, 512], BF16, tag="njrow", bufs=2)
                    nc.vector.tensor_copy(njrow[:], njp[:])
                    dma(p5[3:4, i * 512:(i + 1) * 512], njrow[0:1, :])
                r5 = prp.tile([5, RPC], BF16)
                nc.vector.tensor_copy(r5[0:3, :], pcT[:])
                dma(r5[3:4, :], onesN[0:1, 0:RPC])
                sqc = wp.tile([3, RPC], F32, tag="sq3", bufs=2)
                nc.scalar.activation(sqc[:], pcT[:], AF.Square)
                nqp = pp.tile([1, RPC], F32, space="PSUM", tag="rsp", bufs=1)
                nc.tensor.matmul(out=nqp[:], lhsT=ones3_f[:3, 0:1], rhs=sqc[:],
                                 start=True, stop=True)
                nqrow = wp.tile([1, RPC], BF16, tag="njrow", bufs=2)
                nc.vector.tensor_copy(nqrow[:], nqp[:])
                dma(r5[4:5, :], nqrow[0:1, :])

                # rest of the param DMAs (queued behind the latency-critical few)
                dma(src_d_sb := prp.tile([128, NG], I32, name="srcsb"), src_d[:])
                dma(dst_d_sb := prp.tile([128, NG], I32, name="dstsb"), dst_d[:])
                dma(WB[:], wb_d[:])
                dma(obc[:], obc_d[:])
                dma(xa[:], xa_d[:])
                dma(wina[:], wina_d[:])
                dma(ztab2[:], ztab2_d[:])
                dma(io2[:], io2_d[:])
                dma(oidx[:], oidx_d[:])

                # ---------- d^2 matmuls -> dsq halves (PE + vector MAX) ----------
                HW_ = NT * RPC // 2
                dsq = [prp.tile([128, HW_], BF16, name=f"dsq{i}") for i in range(2)]
                for half in range(2):
                    for ki in range(NT // 2):
                        kt = half * (NT // 2) + ki
                        d2p = pp.tile([128, RPC], F32, space="PSUM", tag="stp", bufs=2)
                        nc.tensor.matmul(out=d2p[:],
                                         lhsT=p5[:, kt * 128:(kt + 1) * 128],
                                         rhs=r5[:], start=True, stop=True)
                        nc.vector.tensor_scalar(out=dsq[half][:, ki * RPC:(ki + 1) * RPC],
                                                in0=d2p[:], scalar1=0.0,
                                                scalar2=None, op0=ALU.max)

                # ---------- degree histogram masks on gpsimd ----------
                iot2 = prp.tile([128, 128], I32)
                nc.gpsimd.iota(iot2[:], pattern=[[0, 2], [1, 64]], base=0,
                               channel_multiplier=0)
                iot2f = prp.tile([128, 128], BF16)
                nc.gpsimd.tensor_copy(iot2f[:], iot2[:])
                io32 = prp.tile([128, 64], I32)
                nc.gpsimd.iota(io32[:], pattern=[[0, 2], [1, 32]], base=0,
                               channel_multiplier=0)
                io32f = prp.tile([128, 64], BF16)
                nc.gpsimd.tensor_copy(io32f[:], io32[:])
                # hist^T [32 quot, 64 rem] per dir via DoubleRow group-pairs
                hps = pp.tile([MAXD, 64], F32, space="PSUM", tag="oat", bufs=1)
                for di, ed in enumerate((src_d_sb, dst_d_sb)):
                    qi = wp.tile([128, NG], I32, tag="qi")
                    ri = wp.tile([128, NG], I32, tag="ri")
                    nc.vector.tensor_scalar(out=qi[:], in0=ed[:], scalar1=6,
                                            scalar2=None,
                                            op0=ALU.logical_shift_right)
                    nc.vector.tensor_scalar(out=ri[:], in0=ed[:], scalar1=63,
                                            scalar2=None, op0=ALU.bitwise_and)
                    qf = wp.tile([128, NG], BF16, tag="qf")
                    rf = wp.tile([128, NG], BF16, tag="rf")
                    nc.vector.tensor_copy(qf[:], qi[:])
                    nc.vector.tensor_copy(rf[:], ri[:])
                    for gp in range(NG // 2):
                        Bt = wp.tile([128, 128], F8, tag="Bt", bufs=3)
                        At = wp.tile([128, 64], F8, tag="At", bufs=3)
                        nc.vector.tensor_tensor(
                            out=Bt[:], in0=iot2f[:],
                            in1=rf[:, 2 * gp:2 * gp + 2].to_broadcast([128, 2, 64]),
                            op=ALU.is_equal)
                        nc.vector.tensor_tensor(
                            out=At[:], in0=io32f[:],
                            in1=qf[:, 2 * gp:2 * gp + 2].to_broadcast([128, 2, 32]),
                            op=ALU.is_equal)
                        for pl in range(2):
                            nc.tensor.matmul(
                                out=hps[di * 32:di * 32 + 32, :],
                                lhsT=At[:, pl * 32:(pl + 1) * 32],
                                rhs=Bt[:, pl * 64:(pl + 1) * 64],
                                start=(gp == 0 and pl == 0),
                                stop=(gp == NG // 2 - 1 and pl == 1),
                                skip_group_check=True)
                hsb = wp.tile([MAXD, 64], F32, tag="hsb", bufs=1)
                nc.scalar.activation(hsb[:], hps[:], AF.Identity)
                dma(ar_in[:, :], hsb[:])
                nc.gpsimd.collective_compute(
                    "AllReduce", ALU.add, replica_groups=[list(range(NC))],
                    ins=[ar_in.opt()], outs=[ar_out.opt()])

                # ---------- EB = exp(A * P(t)) / 16 (poly gaussian basis) ----------
                EBh = [prp.tile([128, HW_], BF16, name=f"EBh{i}") for i in range(2)]
                Ah = [prp.tile([128, HW_], BF16, name=f"Ah{i}") for i in range(2)]
                th = [prp.tile([128, HW_], BF16, name=f"th{i}") for i in range(2)]
                accA = [prp.tile([128, HW_], BF16, name=f"accA{i}") for i in range(2)]
                accB = [prp.tile([128, HW_], BF16, name=f"accB{i}") for i in range(2)]
                for half in range(2):
                    # scalar chain: A = exp(ahalf*d2), d = sqrt(d2), t = exp(kt*d)
                    nc.scalar.activation(Ah[half][:], dsq[half][:], AF.Exp,
                                         scale=ebc[9])
                    nc.scalar.activation(th[half][:], dsq[half][:], AF.Sqrt,
                                         bias=eps12[:, 0:1])
                    nc.scalar.activation(th[half][:], th[half][:], AF.Exp,
                                         scale=ebc[8])
                for half in range(2):
                    # vector Horner: acc = c7*t; acc = (acc + c_s)*t ...
                    nc.vector.tensor_scalar(out=accA[half][:], in0=th[half][:],
                                            scalar1=ebc[7],
                                            scalar2=None, op0=ALU.mult)
                    cur, nxt = accA[half], accB[half]
                    for s in range(6, 0, -1):
                        nc.vector.scalar_tensor_tensor(
                            out=nxt[:], in0=cur[:], scalar=ebc[s],
                            in1=th[half][:], op0=ALU.add, op1=ALU.mult)
                        cur, nxt = nxt, cur
                    # b = (acc + c0) * A
                    nc.vector.scalar_tensor_tensor(
                        out=nxt[:], in0=cur[:], scalar=ebc[0],
                        in1=Ah[half][:], op0=ALU.add, op1=ALU.mult)
                    # EB = exp(b)/16
                    nc.scalar.activation(EBh[half][:], nxt[:], AF.Exp,
                                         bias=ln16c[:, 0:1])
                    for ki in range(NT // 2):
                        kt = half * (NT // 2) + ki
                        dma(EBdup[:, kt * 512:kt * 512 + RPC],
                            EBh[half][:, ki * RPC:(ki + 1) * RPC])
                        dma(EBdup[:, kt * 512 + RPC:(kt + 1) * 512],
                            EBh[half][:, ki * RPC:(ki + 1) * RPC])

                # ---------- h0 + degree embeds (replicated) ----------
                arsb = prp.tile([MAXD, 64], F32)
                dma(arsb[:], ar_out[:, :])
                cmin = prp.tile([MAXD, 64], BF16)
                nc.vector.tensor_scalar(out=cmin[:], in0=arsb[:],
                                        scalar1=float(MAXD - 1), scalar2=None,
                                        op0=ALU.min)
                cntRow = prp.tile([1, 2 * N], BF16)
                dma(cntRow[0:1, 0:N], cmin[0:32, :])
                dma(cntRow[0:1, N:2 * N], cmin[32:64, :])
                hfull = [prp.tile([128, D], F32, name=f"hf{t}") for t in range(NT)]
                for t in range(NT):
                    bcps = pp.tile([128, 128], F32, space="PSUM", tag="psA", bufs=2)
                    for di in range(2):
                        nc.tensor.matmul(out=bcps[di * 64:(di + 1) * 64, :],
                                         lhsT=ones_bf[:1, 0:64],
                                         rhs=cntRow[0:1, di * N + t * 128:
                                                     di * N + (t + 1) * 128],
                                         start=True, stop=True,
                                         skip_group_check=True)
                    ohp = wp.tile([128, 128], BF16, tag="ohp", bufs=2)
                    nc.vector.tensor_scalar(out=ohp[:], in0=bcps[:],
                                            scalar1=io2[:, 0:1],
                                            scalar2=None, op0=ALU.is_equal)
                    hb = pp.tile([128, D], F32, space="PSUM",
                                 tag=("ups0" if t % 2 == 0 else "ups1"), bufs=1)
                    nc.tensor.matmul(out=hb[:], lhsT=xa[:, t * 128:(t + 1) * 128],
                                     rhs=wina[:], start=True, stop=False,
                                     skip_group_check=True)
                    nc.tensor.matmul(out=hb[:], lhsT=ohp[:], rhs=ztab2[:],
                                     start=False, stop=True,
                                     skip_group_check=True)
                    nc.vector.tensor_copy(hfull[t][:], hb[:])
                    dma(htmp_dram[t * 128:(t + 1) * 128, :], hfull[t][:])
                for j in range(2):
                    nc.gpsimd.indirect_dma_start(
                        out=hown[j][:], out_offset=None, in_=htmp_dram[:],
                        in_offset=bass.IndirectOffsetOnAxis(ap=oidx[:, j:j + 1], axis=0))

                # ---------- LN1 @ layer 0 (replicated, 18 tiles) ----------
                srcs = [hfull[t][:] for t in range(NT)] + [hown[j][:] for j in range(2)]
                outs = ([xnT[:, t * 128:(t + 1) * 128] for t in range(NT)]
                        + [xnTo[:, j * 128:(j + 1) * 128] for j in range(2)])
                nT18 = len(srcs)
                var = prp.tile([128, nT18], F32)
                mean = prp.tile([128, nT18], F32)
                for i, s_ap in enumerate(srcs):
                    st6 = wp.tile([128, 6], F32, tag="st6")
                    nc.vector.bn_stats(out=st6[:], in_=s_ap)
                    mv = wp.tile([128, 2], F32, tag="mv")
                    nc.vector.bn_aggr(out=mv[:], in_=st6[:])
                    nc.vector.tensor_copy(mean[:, i:i + 1], mv[:, 0:1])
                    nc.vector.tensor_copy(var[:, i:i + 1], mv[:, 1:2])
                sd = prp.tile([128, nT18], F32)
                nc.scalar.activation(sd[:], var[:], AF.Sqrt, bias=eps5[:, 0:1])
                rstd = prp.tile([128, nT18], F32)
                nc.vector.reciprocal(rstd[:], sd[:])
                for i, s_ap in enumerate(srcs):
                    xn = wp.tile([128, D], BF16, tag="xn")
                    nc.vector.tensor_scalar(out=xn[:], in0=s_ap, scalar1=mean[:, i:i + 1],
                                            scalar2=rstd[:, i:i + 1], op0=ALU.subtract,
                                            op1=ALU.mult)
                    trp = pp.tile([128, 128], BF16, space="PSUM", tag="psA")
                    nc.tensor.transpose(out=trp[:], in_=xn[:], identity=ident[:])
                    nc.vector.tensor_scalar(out=outs[i], in0=trp[:], scalar1=l1g[0],
                                            scalar2=l1b[0], op0=ALU.mult, op1=ALU.add)
                    if i < NT:
                        nc.vector.tensor_scalar(out=xn8[:, i * 128:(i + 1) * 128],
                                                in0=xn[:], scalar1=l1g[0],
                                                scalar2=l1b[0], op0=ALU.mult,
                                                op1=ALU.add)

            # ========================================================
            # LAYERS
            # ========================================================
            def ln_own(srcs, gcol, bcol, outs):
                """LayerNorm on own row-tiles -> transposed bf16 outputs."""
                nT = len(srcs)
                var = wp.tile([128, nT], F32, tag="lnvar")
                mean = wp.tile([128, nT], F32, tag="lnmean")
                for i, s_ap in enumerate(srcs):
                    st6 = wp.tile([128, 6], F32, tag="st6")
                    nc.vector.bn_stats(out=st6[:], in_=s_ap)
                    mv = wp.tile([128, 2], F32, tag="mv")
                    nc.vector.bn_aggr(out=mv[:], in_=st6[:])
                    nc.vector.tensor_copy(mean[:, i:i + 1], mv[:, 0:1])
                    nc.vector.tensor_copy(var[:, i:i + 1], mv[:, 1:2])
                sd = wp.tile([128, nT], F32, tag="lnsd")
                nc.scalar.activation(sd[:, :nT], var[:, :nT], AF.Sqrt, bias=eps5[:, 0:1])
                rstd = wp.tile([128, nT], F32, tag="lnrstd")
                nc.vector.reciprocal(rstd[:, :nT], sd[:, :nT])
                for i, s_ap in enumerate(srcs):
                    xn = wp.tile([128, D], BF16, tag="xn")
                    nc.vector.tensor_scalar(out=xn[:], in0=s_ap, scalar1=mean[:, i:i + 1],
                                            scalar2=rstd[:, i:i + 1], op0=ALU.subtract,
                                            op1=ALU.mult)
                    trp = pp.tile([128, 128], BF16, space="PSUM", tag="psA")
                    nc.tensor.transpose(out=trp[:], in_=xn[:], identity=ident[:])
                    nc.vector.tensor_scalar(out=outs[i], in0=trp[:], scalar1=gcol,
                                            scalar2=bcol, op0=ALU.mult, op1=ALU.add)

            hpt = [wp.tile([128, D], F32, name=f"hp{j}") for j in range(2)]

            for l in range(L):
                # R = (Wk Wq^T) xn_own^T + Wk bq, per head (pre-AllGather work)
                for h in range(H):
                    rp = pp.tile([D, RPC], F32, space="PSUM", tag="psA")
                    nc.tensor.matmul(out=rp[:], lhsT=gt[l][h], rhs=xnTo[:],
                                     start=True, stop=True)
                    nc.vector.tensor_scalar(out=RT[:, h * RPC:(h + 1) * RPC],
                                            in0=rp[:], scalar1=rb[l][h],
                                            scalar2=None, op0=ALU.add)
                # xn8 from gathered xnT (keys in rows, fp8)
                if l > 0:
                    for t in range(NT):
                        trp = pp.tile([128, 128], BF16, space="PSUM", tag="psA")
                        nc.tensor.transpose(out=trp[:],
                                            in_=xnT[:, t * 128:(t + 1) * 128],
                                            identity=ident[:])
                        nc.vector.tensor_copy(xn8[:, t * 128:(t + 1) * 128], trp[:])

                # flash attention, two heads per pass, fp8 DoubleRow PV
                oat = pp.tile([D, RPC], F32, space="PSUM", tag="oat", bufs=1)
                deferred_pe = [None]

                def flush_deferred():
                    if deferred_pe[0] is not None:
                        deferred_pe[0]()
                        deferred_pe[0] = None

                for hp2 in range(H // 2):
                    h0, h1 = 2 * hp2, 2 * hp2 + 1
                    # DoubleRow outputs must sit at partition base 0, so
                    # U[d, q] per head lives as [64, (dh, q)]; the second
                    # column-block group uses start=False and inherits the
                    # bank-wide pending-zero set by the first group's start.
                    Ups = [pp.tile([64, 512], F32, space="PSUM", tag=f"ups{jj}",
                                   bufs=1, name=f"ups{jj}") for jj in range(2)]
                    rsp = pp.tile([32, 512], F32, space="PSUM", tag="rsp", bufs=1)

                    def scores_exp(kt, PT8cur, j):
                        stp = pp.tile([128, 512], F32, space="PSUM", tag="stp",
                                      bufs=2)
                        nc.tensor.matmul(out=stp[:, 0:RPC],
                                         lhsT=xnT[:, kt * 128:(kt + 1) * 128],
                                         rhs=RT[:, h0 * RPC:(h0 + 1) * RPC],
                                         start=True, stop=True,
                                         skip_group_check=True)
                        nc.tensor.matmul(out=stp[:, RPC:2 * RPC],
                                         lhsT=xnT[:, kt * 128:(kt + 1) * 128],
                                         rhs=RT[:, h1 * RPC:(h1 + 1) * RPC],
                                         start=True, stop=True,
                                         skip_group_check=True)
                        PTt = wp.tile([128, 512], BF16, tag="PTt", bufs=3)
                        nc.scalar.activation(PTt[:], stp[:], AF.Exp, scale=SCALE)
                        nc.vector.tensor_tensor(
                            out=PT8cur[:, j * 512:(j + 1) * 512], in0=PTt[:],
                            in1=EBdup[:, kt * 512:(kt + 1) * 512], op=ALU.mult)

                    for t in range(NT // 2):
                        PT8cur = wp.tile([128, 1024], F8 if PT_FP8 else BF16,
                                         tag="PT8", bufs=2)
                        scores_exp(2 * t, PT8cur, 0)
                        scores_exp(2 * t + 1, PT8cur, 1)
                        if t == 0:
                            flush_deferred()
                        p8r = PT8cur[:, :].rearrange("p (j q) -> p j q", j=2)
                        for jj in range(2):
                            rhs = p8r[:, :, jj * RPC:(jj + 1) * RPC]
                            for dh in range(2):
                                nc.tensor.matmul(
                                    out=Ups[jj][0:64, dh * RPC:(dh + 1) * RPC],
                                    lhsT=xn8r[:, 2 * t:2 * t + 2,
                                              dh * 64:(dh + 1) * 64],
                                    rhs=rhs, start=(t == 0 and dh == 0),
                                    stop=(t == NT // 2 - 1),
                                    perf_mode=DR, skip_group_check=True)
                            nc.tensor.matmul(
                                out=rsp[0:32, jj * RPC:(jj + 1) * RPC],
                                lhsT=ones8[:, :].rearrange(
                                    "p (t m) -> p t m", t=2),
                                rhs=rhs,
                                start=(t == 0 and jj == 0),
                                stop=(t == NT // 2 - 1),
                                perf_mode=DR, skip_group_check=True)

                    # epilogue: normalize U per head, apply fused M = Wv Wo
                    rrow = wp.tile([1, 1024], F32, tag="rrow", bufs=2)
                    U8n = wp.tile([64, 1024], BF16, tag="U8n", bufs=2)
                    rbc = wp.tile([64, 1024], F32, tag="rbc", bufs=2)
                    for jj in range(2):
                        # duplicate the reciprocal row for both d-halves
                        nc.vector.reciprocal_approx_fast(
                            out=rrow[0:1, jj * 512:jj * 512 + RPC],
                            in_=rsp[0:1, jj * RPC:(jj + 1) * RPC])
                        nc.vector.tensor_copy(
                            rrow[0:1, jj * 512 + RPC:(jj + 1) * 512],
                            rrow[0:1, jj * 512:jj * 512 + RPC])
                        nc.gpsimd.partition_broadcast(
                            rbc[:, jj * 512:(jj + 1) * 512],
                            rrow[0:1, jj * 512:(jj + 1) * 512])
                        nc.vector.tensor_tensor(
                            out=U8n[:, jj * 512:(jj + 1) * 512],
                            in0=Ups[jj][0:64, :],
                            in1=rbc[:, jj * 512:(jj + 1) * 512], op=ALU.mult)

                    def mk_mapply(hp2=hp2, h0=h0, h1=h1, U8n=U8n):
                        def go():
                            for jj, hh in ((0, h0), (1, h1)):
                                for dh, mwseg in ((0, mwl), (1, mwh)):
                                    nc.tensor.matmul(
                                        out=oat[:],
                                        lhsT=mwseg[l][hh][0:64, :],
                                        rhs=U8n[0:64, jj * 512 + dh * RPC:
                                                jj * 512 + (dh + 1) * RPC],
                                        start=(hp2 == 0 and jj == 0 and dh == 0),
                                        stop=(hp2 == H // 2 - 1 and jj == 1
                                              and dh == 1),
                                        skip_group_check=True)
                        return go
                    deferred_pe[0] = mk_mapply()
                flush_deferred()

                # attention out: + bop, transpose to rows, residual
                oatT = wp.tile([D, RPC], BF16, tag="oatT")
                nc.vector.tensor_scalar(out=oatT[:], in0=oat[:], scalar1=bop[l],
                                        scalar2=None, op0=ALU.add)
                if DEBUG and l == 0:
                    o32 = wp.tile([D, RPC], F32, tag="dbgu", bufs=2)
                    nc.vector.tensor_copy(o32[:], oatT[:])
                    dma(dbg_oatT[:, :], o32[:])
                    x32 = wp.tile([D, RPC], F32, tag="dbgu", bufs=2)
                    for tt in range(NT):
                        nc.vector.tensor_copy(x32[:, 0:128], xnT[:, tt * 128:(tt + 1) * 128])
                        dma(dbg_xnT[:, tt * 128:(tt + 1) * 128], x32[:, 0:128])
                    r32 = wp.tile([D, RPC], F32, tag="dbgu", bufs=2)
                    for hh in range(H):
                        nc.vector.tensor_copy(r32[:], RT[:, hh * RPC:(hh + 1) * RPC])
                        dma(dbg_RT[:, hh * RPC:(hh + 1) * RPC], r32[:])
                for j in range(2):
                    ftp = pp.tile([128, 128], BF16, space="PSUM", tag="psA")
                    nc.tensor.transpose(out=ftp[:], in_=oatT[:, j * 128:(j + 1) * 128],
                                        identity=ident[:])
                    nc.vector.tensor_tensor(out=hpt[j][:], in0=hown[j][:],
                                            in1=ftp[:], op=ALU.add)
                # LN2 + FF on own rows
                ln_own([hpt[j][:] for j in range(2)], l2g[l], l2b[l],
                       [xn2T[:, j * 128:(j + 1) * 128] for j in range(2)])
                for fs in range(4):
                    fp = pp.tile([128, RPC], F32, space="PSUM", tag="psA")
                    nc.tensor.matmul(out=fp[:], lhsT=w1[l][:, fs * 128:(fs + 1) * 128],
                                     rhs=xn2T[:], start=True, stop=True)
                    nc.scalar.activation(gT[fs][:], fp[:],
                                         AF.Gelu, bias=b1c[l][fs])
                fdp = pp.tile([D, RPC], F32, space="PSUM", tag="stp", bufs=2)
                for fs in range(4):
                    nc.tensor.matmul(out=fdp[:], lhsT=w2[l][fs],
                                     rhs=gT[fs][:],
                                     start=(fs == 0), stop=(fs == 3))
                ffdT = wp.tile([D, RPC], BF16, tag="ffdT")
                nc.vector.tensor_scalar(out=ffdT[:], in0=fdp[:], scalar1=b2c[l],
                                        scalar2=None, op0=ALU.add)
                for j in range(2):
                    ftp = pp.tile([128, 128], BF16, space="PSUM", tag="psA")
                    nc.tensor.transpose(out=ftp[:], in_=ffdT[:, j * 128:(j + 1) * 128],
                                        identity=ident[:])
                    nc.vector.tensor_tensor(out=hown[j][:], in0=hpt[j][:], in1=ftp[:],
                                            op=ALU.add)
                if DEBUG and l == 0:
                    for j in range(2):
                        dma(dbg_h1[:, j * D:(j + 1) * D], hown[j][:])
                # LN1 for next layer on own rows; AllGather xn^T (bf16)
                if l < L - 1:
                    ln_own([hown[j][:] for j in range(2)], l1g[l + 1], l1b[l + 1],
                           [xnTo[:, j * 128:(j + 1) * 128] for j in range(2)])
                    dma(ag_in[l][:, :], xnTo[:])
                    nc.gpsimd.collective_compute(
                        "AllGather", ALU.bypass, replica_groups=[list(range(NC))],
                        ins=[ag_in[l].opt()], outs=[ag_out[l].opt()])
                    for c in range(NC):
                        dma(xnT[:, c * RPC:(c + 1) * RPC],
                            ag_out[l][c * D:(c + 1) * D, :])

            # ================= output =================
            for j in range(2):
                hb16 = wp.tile([128, D], BF16, tag="hb16")
                nc.vector.tensor_copy(hb16[:], hown[j][:])
                htp = pp.tile([128, 128], BF16, space="PSUM", tag="psA")
                nc.tensor.transpose(out=htp[:], in_=hb16[:], identity=ident[:])
                hT = wp.tile([D, 128], BF16, tag="hT")
                nc.vector.tensor_copy(hT[:], htp[:])
                op_ps = pp.tile([OUT, 128], F32, space="PSUM", tag="psA")
                nc.tensor.matmul(out=op_ps[:], lhsT=ow, rhs=hT[:],
                                 start=True, stop=True)
                ob_sb = wp.tile([OUT, 128], F32, tag="ob_sb")
                nc.scalar.activation(ob_sb[:], op_ps[:], AF.Identity,
                                     bias=obc[:, 0:1])
                dma(out_d[:, j * 128:(j + 1) * 128], ob_sb[:])

    nc.finalize()
    return nc


def _pack_bf16(I):
    Wq, Wk, Wo_, Wv_ = f32(I["Wq"]), f32(I["Wk"]), f32(I["Wo"]), f32(I["Wv"])
    cols = []
    for l in range(L):
        for h in range(H):
            cols.append(Wq[l, h] @ Wk[l, h].T)          # gt: lhsT for R
    mwfull = [[Wv_[l, h] @ Wo_[l, h * D:(h + 1) * D, :] for h in range(H)]
              for l in range(L)]
    z64 = np.zeros((64, D), np.float32)
    for l in range(L):
        for h in range(H):
            cols.append(np.concatenate([mwfull[l][h][0:64], z64], 0))   # mwl
    for l in range(L):
        for h in range(H):
            cols.append(np.concatenate([mwfull[l][h][64:128], z64], 0))  # mwh
    for l in range(L):
        cols.append(f32(I["ff1_w"])[l])
    for l in range(L):
        for fs in range(4):
            cols.append(f32(I["ff2_w"])[l, fs * 128:(fs + 1) * 128, :])
    cols.append(f32(I["out_w"]))
    out = np.concatenate(cols, 1)
    assert out.shape == (128, NBF), out.shape
    return bf16(out)


def _pack_f32(I):
    Wk, Wo_ = f32(I["Wk"]), f32(I["Wo"])
    cols = []
    for l in range(L):
        for h in range(H):
            cols.append((Wk[l, h] @ f32(I["bq"])[l, h])[:, None])  # rb
    for nm in ("ln1_g", "ln1_b", "ln2_g", "ln2_b"):
        for l in range(L):
            cols.append(f32(I[nm])[l][:, None])
    for l in range(L):
        cols.append(f32(I["ff2_b"])[l][:, None])
    for l in range(L):
        for fs in range(4):
            cols.append(f32(I["ff1_b"])[l, fs * 128:(fs + 1) * 128][:, None])
    for l in range(L):
        bo_l = f32(I["bo"])[l] + sum(
            f32(I["bv"])[l, h] @ Wo_[l, h * D:(h + 1) * D, :] for h in range(H))
        cols.append(bo_l[:, None])
    # EB poly constants: c0..c7, kt, ahalf (broadcast down partitions)
    mu = f32(I["sp_mu"]); sg = f32(I["sp_sigma"]); w = f32(I["sp_w"])
    s2 = float(sg[0]) ** 2
    step = float(mu[1] - mu[0])
    c = w * np.exp(-0.5 * mu * mu / s2)
    ebvals = list(c.astype(np.float64)) + [step / s2, -0.5 / s2]
    for v in ebvals:
        cols.append(np.full((128, 1), v, np.float32))
    out = np.concatenate(cols, 1)
    assert out.shape == (128, NF32), out.shape
    return f32(out)


def _prep(inputs):
    I = {k: np.asarray(v) for k, v in inputs.items()}
    x = f32(I["x"])
    pos = f32(I["pos"])
    ei = I["edge_index"].astype(np.int32)

    common = {
        "xa": bf16(np.concatenate([x.T, np.ones((1, N), np.float32)], 0)),
        "wina": bf16(np.concatenate([f32(I["node_in_w"]), f32(I["node_in_b"])[None]], 0)),
        "ztab2": bf16(np.concatenate([f32(I["z_out"]), f32(I["z_in"])], 0)),
        "posT": bf16(pos.T),
        "wb": _pack_bf16(I),
        "pf": _pack_f32(I),
        "obc": f32(I["out_b"])[:, None],
        "io2": np.concatenate([np.arange(MAXD, dtype=np.float32)] * 2)[:, None],
    }
    common = {k: np.ascontiguousarray(v) for k, v in common.items()}
    in_maps = []
    for c in range(NC):
        m = dict(common)
        qsl = slice(c * RPC, (c + 1) * RPC)
        m["pcT"] = bf16(pos[qsl].T)
        m["src"] = np.ascontiguousarray(
            ei[0, c * EPC:(c + 1) * EPC].reshape(NG, 128).T)
        m["dst"] = np.ascontiguousarray(
            ei[1, c * EPC:(c + 1) * EPC].reshape(NG, 128).T)
        m["oidx"] = np.ascontiguousarray(
            (c * RPC + np.arange(RPC, dtype=np.int32)).reshape(2, 128).T)
        in_maps.append(m)
    return in_maps


def kernel(**inputs) -> np.ndarray:
    if "nc" not in _cached:
        _cached["nc"] = build()
    in_maps = _prep(inputs)
    res = run_bass_kernel_spmd(_cached["nc"], in_maps, core_ids=list(range(NC)))
    _cached["last_results"] = res
    out = np.concatenate([f32(r["out"]).T for r in res.results], 0)
    return out.astype(np.float32)
